# revision 1
# baseline (speedup 1.0000x reference)
"""Multi-head causal attention (B=4, S=2048, D=768, H=4 heads) on 8 TRN2 cores.

Sharding: core c handles batch b = c//2 and head-pair hp = c%2 (heads 2*hp,
2*hp+1).  Each core projects x[b] through its 384-column slice of Wq/Wk/Wv,
runs causal attention for its two heads, and pushes the result through its
384-row slice of Wo.  The host sums the two partial outputs per batch and
adds bo.  This splits every matmul's FLOPs exactly 8 ways with no duplicated
work and needs no device collectives.

Dataflow is kept transposed end-to-end ([feature, seq] layouts) so the kernel
needs zero on-device transposes:
  QT/KT = W^T x^T           [384, S]   (3 chunks of 128 partitions)
  V     = x W               [S, 384]   (16 chunks of 128 partitions, with a
                                        ones column appended per head so the
                                        softmax denominator falls out of the
                                        ctx matmul as one extra output row)
  S^T   = KT'Q              [k, q]     k on partitions -> softmax sum over k
  ctx^T = V^T E             [192+1, q]
  out^T = Wo^T ctx^T        [768, S]
Causal structure: key-tile i (128 rows) x query-tile j (512 cols) blocks with
i > 4j+3 are fully masked and skipped entirely; diagonal blocks get a 0/1
mask multiply after exp.  Scores are O(1) so exp needs no max-subtraction.

The query-tile loop is software-pipelined so the (DVE-heavy) softmax
normalization of tile j-1 overlaps the (PE-heavy) score phase of tile j,
keeping TensorE dense and the HAM clock-gate warm.

Matmul operands are fp16 (PSUM accumulates fp32).
"""

import sys

for _p in ("/opt/trn_rl_repo",):
    if _p not in sys.path:
        sys.path.insert(0, _p)

import numpy as np

S = 2048            # sequence length
D = 768             # model dim
DH = 192            # head dim
DD = 2 * DH         # feature columns per core (2 heads)
P = 128             # partitions
KC = D // P         # 6 contraction chunks over D
MC = DD // P        # 3 chunks over the per-core head dims
QT = 512            # query tile (matmul free dim, one PSUM bank)
NQ = S // QT        # 4 query tiles
NK = S // P         # 16 key tiles
SCALE = 1.0 / float(np.sqrt(DH))

# Per-head slices of the [384 -> 3x128chunk] QT/KT layout, ordered so the two
# K=64 pieces of the two heads land in different PE row groups (base partition
# 0 vs 64) and can overlap in the array.
#   h=0: chunk0 rows 0:128  +  chunk1 rows 0:64
#   h=1: chunk2 rows 0:128  +  chunk1 rows 64:128
HEAD_PIECES = [
    [(0, 0, 128), (1, 0, 64)],
    [(2, 0, 128), (1, 64, 64)],
]

_CACHE = {}


def _build_nc():
    import concourse.bacc as bacc
    import concourse.tile as tile
    from concourse import mybir

    F16 = mybir.dt.float16
    F32 = mybir.dt.float32
    EXP = mybir.ActivationFunctionType.Exp
    IDENT = mybir.ActivationFunctionType.Identity

    nc = bacc.Bacc(None, target_bir_lowering=False)

    xt = nc.dram_tensor("xt", [P, KC, S], F16, kind="ExternalInput")
    wq = nc.dram_tensor("wq", [P, KC, DD], F16, kind="ExternalInput")
    wk = nc.dram_tensor("wk", [P, KC, DD], F16, kind="ExternalInput")
    wv = nc.dram_tensor("wv", [P, KC, DD], F16, kind="ExternalInput")
    wo = nc.dram_tensor("wo", [P, 4, D], F16, kind="ExternalInput")
    bqk = nc.dram_tensor("bqk", [P, 6], F32, kind="ExternalInput")
    bvr = nc.dram_tensor("bvr", [1, DD], F32, kind="ExternalInput")
    msk = nc.dram_tensor("msk", [P, 4, QT], F16, kind="ExternalInput")
    out_t = nc.dram_tensor("out_t", [P, KC, S], F32, kind="ExternalOutput")

    with tile.TileContext(nc) as tc:
        with (
            tc.tile_pool(name="persist", bufs=1) as pp,
            tc.tile_pool(name="epool", bufs=56) as ep,
            tc.tile_pool(name="ctxp", bufs=3) as cp,
            tc.tile_pool(name="workp", bufs=2) as wp,
            tc.tile_pool(name="outp", bufs=3) as op_,
            tc.tile_pool(name="psA", bufs=3, space="PSUM") as psA,
            tc.tile_pool(name="psC", bufs=2, space="PSUM") as psC,
        ):
            # ---- loads, split across both HWDGE rings (sync + scalar).
            # The critical path is wq + the first x quarter (the first QT
            # matmuls); those go first, with the x quarter split across both
            # rings so it finishes soonest.
            x_sb = pp.tile([P, KC, S], F16)
            wq_sb = pp.tile([P, KC, DD], F16)
            wk_sb = pp.tile([P, KC, DD], F16)
            nc.sync.dma_start(out=x_sb[:, 0:3, 0:QT], in_=xt[:, 0:3, 0:QT])
            nc.scalar.dma_start(out=x_sb[:, 3:6, 0:QT], in_=xt[:, 3:6, 0:QT])
            nc.sync.dma_start(out=wq_sb, in_=wq[:, :, :])
            nc.scalar.dma_start(out=wk_sb, in_=wk[:, :, :])
            nc.sync.dma_start(out=x_sb[:, :, QT : 2 * QT], in_=xt[:, :, QT : 2 * QT])
            bqk_sb = pp.tile([P, 6], F32)
            nc.scalar.dma_start(out=bqk_sb, in_=bqk[:, :])
            bvr_sb = pp.tile([1, DD], F32)
            nc.scalar.dma_start(out=bvr_sb, in_=bvr[:, :])
            nc.sync.dma_start(
                out=x_sb[:, :, 2 * QT : 3 * QT], in_=xt[:, :, 2 * QT : 3 * QT]
            )
            wv_sb = pp.tile([P, KC, DD], F16)
            nc.scalar.dma_start(out=wv_sb, in_=wv[:, :, :])
            nc.sync.dma_start(out=x_sb[:, :, 3 * QT : S], in_=xt[:, :, 3 * QT : S])
            wo_sb = pp.tile([P, 4, D], F16)
            nc.scalar.dma_start(out=wo_sb, in_=wo[:, :, :])
            msk_sb = pp.tile([P, 4, QT], F16)
            nc.scalar.dma_start(out=msk_sb, in_=msk[:, :, :])

            ones_sb = pp.tile([1, P], F16)
            nc.vector.memset(ones_sb, 1.0)

            # Dummy matmuls on the just-memset ones tile: keeps TensorE busy
            # during the initial loads so the HAM clock-gate is already warm
            # (2.4 GHz) when the first real matmul issues.
            ps_warm = psA.tile([P, QT], F32, tag="mm", name="ps_warm")
            for w in range(64):
                nc.tensor.matmul(
                    ps_warm[:, 0:P], lhsT=ones_sb, rhs=ones_sb,
                    start=True, stop=True, skip_group_check=(w > 0),
                )

            # ---- Q^T and K^T projections: [384(3x128), 2048] fp16 ----
            # (bias-add + fp16 cast on ScalarE to keep DVE free)
            qt_sb = pp.tile([P, MC, S], F16)
            kt_sb = pp.tile([P, MC, S], F16)
            for s in range(NQ):
                for t, (w_sb, dst, boff) in enumerate(
                    [(wq_sb, qt_sb, 0), (wk_sb, kt_sb, 3)]
                ):
                    for m in range(MC):
                        ps = psA.tile(
                            [P, QT], F32, tag="mm", name=f"psp{t}_{m}_{s}"
                        )
                        for c in range(KC):
                            nc.tensor.matmul(
                                ps,
                                lhsT=w_sb[:, c, m * P : (m + 1) * P],
                                rhs=x_sb[:, c, s * QT : (s + 1) * QT],
                                start=(c == 0),
                                stop=(c == KC - 1),
                            )
                        nc.scalar.activation(
                            dst[:, m, s * QT : (s + 1) * QT],
                            ps,
                            IDENT,
                            bias=bqk_sb[:, boff + m : boff + m + 1],
                        )

            # V bias broadcast to all partitions: bvb[p, n] = bv[n]
            bvb_sb = pp.tile([P, DD], F32)
            nc.gpsimd.partition_broadcast(bvb_sb, bvr_sb)

            # ---- attention state ----
            e_store = {}   # j -> E tiles
            cx_store = {}  # j -> (cA, cB) ctx psums
            rd_store = {}  # j -> 1/denom fp16 rows
            ct_store = {}  # j -> normalized fp16 ctx tiles
            mask_q = []    # deferred (e, r) causal-mask multiplies

            def emit_scores_i(j, i, qs):
                """Scores + exp for key tile i of q-tile j.  The causal mask
                multiply is deferred (flush_masks) and runs on GpSimd so it
                stays off both the ACT and DVE critical queues."""
                ks = slice(i * P, (i + 1) * P)
                sps = [
                    psA.tile([P, QT], F32, tag="mm", name=f"sc{h}_{j}_{i}")
                    for h in range(2)
                ]
                # K=128 pieces, then the two K=64 pieces back-to-back
                # (different PE row groups -> they overlap in the array)
                for pi in range(2):
                    for h in range(2):
                        c, p0, pl = HEAD_PIECES[h][pi]
                        nc.tensor.matmul(
                            sps[h],
                            lhsT=kt_sb[p0 : p0 + pl, c, ks],
                            rhs=qt_sb[p0 : p0 + pl, c, qs],
                            start=(pi == 0),
                            stop=(pi == 1),
                        )
                r = i - 4 * j
                pair = []
                for h in range(2):
                    e = ep.tile([P, QT], F16, tag="e", name=f"e{h}_{j}_{i}")
                    nc.scalar.activation(e, sps[h], EXP, scale=SCALE)
                    if r >= 0:
                        mask_q.append((e, r))
                    pair.append(e)
                return pair

            def flush_masks():
                while mask_q:
                    e, r = mask_q.pop(0)
                    nc.vector.tensor_mul(e, e, msk_sb[:, r, :])

            # ---- V projection, seq-major, ones column at col 128 ----
            # v_sb[:, i, h, :] = [V_d0:128 | ones | V_d128:192]; ctx piece A =
            # cols 0:128, piece B = cols 128:193 (denominator row 0 + 64 V).
            # Query tiles are processed in DESCENDING order, so the longest
            # score phase (j=3, 16 key tiles) interleaves 1:1 with this loop,
            # and every later score phase hides behind a longer ctx phase.
            v_sb = pp.tile([P, NK, 2, DH + 1], F16)
            nc.gpsimd.memset(v_sb[:, :, :, 128:129], 1.0)
            ets3 = []
            qs3 = slice(3 * QT, 4 * QT)
            for i in range(NK):
                ps = psA.tile([P, QT], F32, tag="mm", name=f"psv{i}")
                for c in range(KC):
                    nc.tensor.matmul(
                        ps[:, 0:DD],
                        lhsT=x_sb[:, c, i * P : (i + 1) * P],
                        rhs=wv_sb[:, c, :],
                        start=(c == 0),
                        stop=(c == KC - 1),
                    )
                for h in range(2):
                    nc.vector.tensor_add(
                        v_sb[:, i, h, 0:128],
                        ps[:, h * DH : h * DH + 128],
                        bvb_sb[:, h * DH : h * DH + 128],
                    )
                    nc.vector.tensor_add(
                        v_sb[:, i, h, 129 : DH + 1],
                        ps[:, h * DH + 128 : (h + 1) * DH],
                        bvb_sb[:, h * DH + 128 : (h + 1) * DH],
                    )
                ets3.append(emit_scores_i(3, i, qs3))
            e_store[3] = ets3
            flush_masks()

            # ---- attention + output projection, software-pipelined ----
            def alloc_ctx(j):
                cA = [
                    psC.tile([P, QT], F32, tag="cA", name=f"cA{h}_{j}", bufs=2)
                    for h in range(2)
                ]
                cB = [
                    psC.tile([65, QT], F32, tag="cB", name=f"cB{h}_{j}", bufs=3)
                    for h in range(2)
                ]
                cx_store[j] = (cA, cB)
                return cA, cB

            def emit_recip(j):
                """1/denom (DVE) broadcast to all partitions (GpSimd).
                Emitted right after the cB chains stop so the whole chain
                overlaps the rest of the block."""
                _, cB = cx_store[j]
                bsbs = []
                for h in range(2):
                    rd = wp.tile([1, QT], F32, tag="rd", name=f"rd{h}_{j}")
                    nc.vector.reciprocal(rd, cB[h][0:1, :])
                    bsb = wp.tile([P, QT], F32, tag="bsb", name=f"bsb{h}_{j}")
                    nc.gpsimd.partition_broadcast(bsb, rd)
                    bsbs.append(bsb)
                rd_store[j] = bsbs

            def emit_finish(j):
                """Scale ctx by the broadcast 1/denom (DVE)."""
                cA, cB = cx_store.pop(j)
                bsbs = rd_store.pop(j)
                ctxs = []
                for h in range(2):
                    cta = cp.tile([P, QT], F16, tag="ctA", name=f"ctA{h}_{j}")
                    nc.vector.tensor_mul(cta, cA[h], bsbs[h])
                    ctb = cp.tile([65, QT], F16, tag="ctB", name=f"ctB{h}_{j}")
                    nc.vector.tensor_mul(ctb, cB[h][0:65, :], bsbs[h][0:65, :])
                    ctxs.append((cta, ctb))
                ct_store[j] = ctxs

            def emit_outproj_m(j, ms_list):
                """Output projection + store for query tile j, given m chunks."""
                qs = slice(j * QT, (j + 1) * QT)
                (ctA0, ctB0), (ctA1, ctB1) = ct_store[j]
                for m in ms_list:
                    ms = slice(m * P, (m + 1) * P)
                    po = psA.tile([P, QT], F32, tag="mm", name=f"po{m}_{j}")
                    nc.tensor.matmul(po, lhsT=wo_sb[:, 0, ms], rhs=ctA0, start=True, stop=False)
                    nc.tensor.matmul(po, lhsT=wo_sb[0:65, 1, ms], rhs=ctB0, start=False, stop=False)
                    nc.tensor.matmul(po, lhsT=wo_sb[:, 2, ms], rhs=ctA1, start=False, stop=False)
                    nc.tensor.matmul(po, lhsT=wo_sb[0:65, 3, ms], rhs=ctB1, start=False, stop=True)
                    osb = op_.tile([P, QT], F32, tag="osb", name=f"osb{m}_{j}")
                    nc.vector.tensor_copy(osb, po)
                    nc.sync.dma_start(out=out_t[:, m, qs], in_=osb)

            def emit_cb_chain(jc, ec, cB):
                nk_c = 4 * jc + 4
                for i in range(nk_c):
                    for h in range(2):
                        nc.tensor.matmul(
                            cB[h],
                            lhsT=v_sb[:, i, h, 128 : DH + 1],
                            rhs=ec[i][h],
                            start=(i == 0),
                            stop=(i == nk_c - 1),
                        )

            def emit_ca_i(jc, ec, cA, i):
                nk_c = 4 * jc + 4
                for h in range(2):
                    nc.tensor.matmul(
                        cA[h],
                        lhsT=v_sb[:, i, h, 0:128],
                        rhs=ec[i][h],
                        start=(i == 0),
                        stop=(i == nk_c - 1),
                    )

            def emit_block(js):
                """Score phase of q-tile js with the ctx of q-tile js+1
                interleaved, plus the out-projection of q-tile js+2.  The cB
                (denominator) chains go first so the reciprocal + partition
                broadcast overlap the block; ctx is always 4 key-tiles
                longer than the score phase, so the PE never waits on ACT
                exps, and the finish muls complete during the ctx tail."""
                jc = js + 1
                nk_s = 4 * js + 4
                nk_c = 4 * jc + 4
                qs = slice(js * QT, (js + 1) * QT)
                ec = e_store.pop(jc)
                cA, cB = alloc_ctx(jc)
                emit_cb_chain(jc, ec, cB)
                if jc + 1 in ct_store:
                    emit_outproj_m(jc + 1, list(range(KC)))
                    ct_store.pop(jc + 1)
                emit_recip(jc)
                ets = []
                for i in range(nk_s):
                    emit_ca_i(jc, ec, cA, i)
                    ets.append(emit_scores_i(js, i, qs))
                for i in range(nk_s, nk_c):
                    emit_ca_i(jc, ec, cA, i)
                emit_finish(jc)
                e_store[js] = ets
                flush_masks()

            for js in range(NQ - 2, -1, -1):  # 2, 1, 0
                emit_block(js)
            # epilogue: ctx(0) (4 key tiles) + the two pending projections.
            e0 = e_store.pop(0)
            cA, cB = alloc_ctx(0)
            emit_cb_chain(0, e0, cB)
            emit_outproj_m(1, list(range(KC)))
            ct_store.pop(1)
            emit_recip(0)
            for i in range(4):
                emit_ca_i(0, e0, cA, i)
            emit_finish(0)
            emit_outproj_m(0, list(range(KC)))
            ct_store.pop(0)

    nc.compile()
    return nc


def _get_nc():
    if "nc" not in _CACHE:
        _CACHE["nc"] = _build_nc()
    return _CACHE["nc"]


def _masks():
    kk = np.arange(P)[:, None, None]
    r = np.arange(4)[None, :, None]
    qq = np.arange(QT)[None, None, :]
    return (qq >= kk + P * r).astype(np.float16)


def host_prep(x, Wq, bq, Wk, bk, Wv, bv, Wo):
    """Build the 8 per-core input maps (core c: batch c//2, head-pair c%2)."""
    f16 = np.float16
    x = np.asarray(x, dtype=np.float32)
    Wq, Wk, Wv, Wo = (np.asarray(a, dtype=np.float32) for a in (Wq, Wk, Wv, Wo))
    bq, bk, bv = (np.asarray(a, dtype=np.float32) for a in (bq, bk, bv))
    masks = _masks()
    xt16 = {}
    for b in range(4):
        xt16[b] = np.ascontiguousarray(
            x[b].T.reshape(KC, P, S).transpose(1, 0, 2)
        ).astype(f16)
    in_maps = []
    for c in range(8):
        b, hp = divmod(c, 2)
        cs = slice(hp * DD, (hp + 1) * DD)
        wq16 = np.ascontiguousarray(
            Wq[:, cs].reshape(KC, P, DD).transpose(1, 0, 2)
        ).astype(f16)
        wk16 = np.ascontiguousarray(
            Wk[:, cs].reshape(KC, P, DD).transpose(1, 0, 2)
        ).astype(f16)
        wv16 = np.ascontiguousarray(
            Wv[:, cs].reshape(KC, P, DD).transpose(1, 0, 2)
        ).astype(f16)
        wo_s = Wo[cs, :]
        woc = np.zeros((P, 4, D), np.float32)
        woc[:, 0, :] = wo_s[0:128]
        woc[1:65, 1, :] = wo_s[128:192]
        woc[:, 2, :] = wo_s[192:320]
        woc[1:65, 3, :] = wo_s[320:384]
        bqk_c = np.concatenate(
            [bq[cs].reshape(MC, P).T, bk[cs].reshape(MC, P).T], axis=1
        ).astype(np.float32)
        in_maps.append(
            {
                "xt": xt16[b],
                "wq": wq16,
                "wk": wk16,
                "wv": wv16,
                "wo": woc.astype(f16),
                "bqk": np.ascontiguousarray(bqk_c),
                "bvr": np.ascontiguousarray(bv[cs].reshape(1, DD)).astype(np.float32),
                "msk": masks,
            }
        )
    return in_maps


def combine(per_core_out, bo):
    """Sum the per-batch core pairs and undo the transposed layout."""
    bo = np.asarray(bo, dtype=np.float32)
    out = np.empty((4, S, D), np.float32)
    for b in range(4):
        pt = per_core_out[2 * b] + per_core_out[2 * b + 1]  # [P, KC, S]
        out[b] = pt.transpose(1, 0, 2).reshape(D, S).T + bo
    return out


def run(inp, trace=False):
    from concourse.bass_utils import run_bass_kernel_spmd

    nc = _get_nc()
    in_maps = host_prep(
        inp["inputs"], inp["Wq"], inp["bq"], inp["Wk"], inp["bk"],
        inp["Wv"], inp["bv"], inp["Wo"],
    )
    kw = {}
    if trace:
        kw = dict(trace=True, trace_cores=list(range(8)))
    res = run_bass_kernel_spmd(nc, in_maps, core_ids=list(range(8)), **kw)
    out = combine([r["out_t"] for r in res.results], inp["bo"])
    return out, res


def kernel(inputs, Wq, bq, Wk, bk, Wv, bv, Wo, bo):
    out, _ = run(
        {"inputs": inputs, "Wq": Wq, "bq": bq, "Wk": Wk, "bk": bk,
         "Wv": Wv, "bv": bv, "Wo": Wo, "bo": bo}
    )
    return out



# revision 2
# speedup vs baseline: 1.0255x; 1.0255x over previous
"""Multi-head causal attention (B=4, S=2048, D=768, H=4 heads) on 8 TRN2 cores.

Sharding: core c handles batch b = c//2 and head-pair hp = c%2 (heads 2*hp,
2*hp+1).  Each core projects x[b] through its 384-column slice of Wq/Wk/Wv,
runs causal attention for its two heads, and pushes the result through its
384-row slice of Wo.  The host sums the two partial outputs per batch and
adds bo.  This splits every matmul's FLOPs exactly 8 ways with no duplicated
work and needs no device collectives.

Dataflow is kept transposed end-to-end ([feature, seq] layouts) so the kernel
needs zero on-device transposes:
  QT = Wq^T x^T     [384, S]    3 chunks of 128 partitions
  KT = Wk^T x^T     [512, S]    4 chunks; the per-head 64-dim remainders are
                                zero-padded to full 128-row chunks so every
                                score matmul is a dense K=128 x N=512 (the
                                mixed-K pipeline stalls ~100ns per K=64 op)
  V   = x W         [S, 384]    seq-major, a ones column per head so the
                                softmax denominator falls out of the ctx
                                matmul as one extra output row
  S^T = KT'Q        [k, q]      k on partitions -> softmax sum over k
  ctx^T = V^T E     [128+65, q]
  out^T = Wo^T ctx^T [768, S]   the two heads' 64-dim ctx remainders are
                                packed into one [128, S] tile so the output
                                projection is 3 dense K=128 matmuls per chunk
Causal structure: key-tile i (128 rows) x query-tile j (512 cols) blocks with
i > 4j+3 are fully masked and skipped entirely; diagonal blocks get a 0/1
mask multiply (on the Pool engine) right after exp.  Scores are O(1) so exp
needs no max-subtraction.

Engine balance: PE does matmuls only; ACT does exp, the QK bias casts, the
PSUM->SBUF output copies and the denominator-row extracts; DVE does fp16
reciprocals, the V bias adds and the ctx normalize muls; Pool does the causal
masks and the reciprocal partition-broadcasts.  The query-tile loop is
software-pipelined so each tile's softmax normalization overlaps the next
tile's score phase.

Matmul operands are fp16 (PSUM accumulates fp32); outputs return fp16.
"""

import sys

for _p in ("/opt/trn_rl_repo",):
    if _p not in sys.path:
        sys.path.insert(0, _p)

import numpy as np

S = 2048            # sequence length
D = 768             # model dim
DH = 192            # head dim
DD = 2 * DH         # feature columns per core (2 heads)
P = 128             # partitions
KC = D // P         # 6 contraction chunks over D
QT = 512            # query tile (matmul free dim, one PSUM bank)
NQ = S // QT        # 4 query tiles
NK = S // P         # 16 key tiles
SCALE = 1.0 / float(np.sqrt(DH))

_CACHE = {}


def _build_nc():
    import concourse.bacc as bacc
    import concourse.tile as tile
    from concourse import mybir

    F16 = mybir.dt.float16
    F32 = mybir.dt.float32
    EXP = mybir.ActivationFunctionType.Exp
    IDENT = mybir.ActivationFunctionType.Identity

    nc = bacc.Bacc(None, target_bir_lowering=False)

    xt = nc.dram_tensor("xt", [P, KC, S], F16, kind="ExternalInput")
    wq = nc.dram_tensor("wq", [P, KC, DD], F16, kind="ExternalInput")
    wk = nc.dram_tensor("wk", [P, KC, DD], F16, kind="ExternalInput")
    wv = nc.dram_tensor("wv", [P, KC, DD], F16, kind="ExternalInput")
    wo = nc.dram_tensor("wo", [P, 3, D], F16, kind="ExternalInput")
    bqk = nc.dram_tensor("bqk", [P, 6], F32, kind="ExternalInput")
    bvr = nc.dram_tensor("bvr", [1, DD], F32, kind="ExternalInput")
    msk = nc.dram_tensor("msk", [P, 4, QT], F16, kind="ExternalInput")
    out_t = nc.dram_tensor("out_t", [P, KC, S], F16, kind="ExternalOutput")

    with tile.TileContext(nc) as tc:
        with (
            tc.tile_pool(name="persist", bufs=1) as pp,
            tc.tile_pool(name="epool", bufs=56) as ep,
            tc.tile_pool(name="ctxp", bufs=6) as cp,
            tc.tile_pool(name="workp", bufs=3) as wp,
            tc.tile_pool(name="outp", bufs=3) as op_,
            tc.tile_pool(name="psA", bufs=3, space="PSUM") as psA,
            tc.tile_pool(name="psC", bufs=2, space="PSUM") as psC,
        ):
            # ---- loads, split across both HWDGE rings (sync + scalar).
            # Ring order is matched to first-use times: wq + the first x
            # quarter gate the first projection matmuls; each later x
            # quarter is split across both rings.
            x_sb = pp.tile([P, KC, S], F16)
            wq_sb = pp.tile([P, KC, DD], F16)
            wk_sb = pp.tile([P, KC, DD], F16)
            wv_sb = pp.tile([P, KC, DD], F16)
            wo_sb = pp.tile([P, 3, D], F16)
            bqk_sb = pp.tile([P, 6], F32)
            bvr_sb = pp.tile([1, DD], F32)
            msk_sb = pp.tile([P, 4, QT], F16)

            nc.sync.dma_start(out=wq_sb, in_=wq[:, :, :])
            nc.scalar.dma_start(out=x_sb[:, 3:6, 0:QT], in_=xt[:, 3:6, 0:QT])
            nc.sync.dma_start(out=x_sb[:, 0:3, 0:QT], in_=xt[:, 0:3, 0:QT])
            nc.scalar.dma_start(out=wk_sb, in_=wk[:, :, :])
            nc.sync.dma_start(
                out=x_sb[:, 0:3, QT : 2 * QT], in_=xt[:, 0:3, QT : 2 * QT]
            )
            nc.scalar.dma_start(
                out=x_sb[:, 3:6, QT : 2 * QT], in_=xt[:, 3:6, QT : 2 * QT]
            )
            nc.scalar.dma_start(out=bqk_sb, in_=bqk[:, :])
            nc.scalar.dma_start(out=bvr_sb, in_=bvr[:, :])
            nc.sync.dma_start(
                out=x_sb[:, 0:3, 2 * QT : 3 * QT], in_=xt[:, 0:3, 2 * QT : 3 * QT]
            )
            nc.scalar.dma_start(
                out=x_sb[:, 3:6, 2 * QT : 3 * QT], in_=xt[:, 3:6, 2 * QT : 3 * QT]
            )
            nc.scalar.dma_start(out=wv_sb, in_=wv[:, :, :])
            nc.sync.dma_start(out=x_sb[:, 0:3, 3 * QT : S], in_=xt[:, 0:3, 3 * QT : S])
            nc.scalar.dma_start(
                out=x_sb[:, 3:6, 3 * QT : S], in_=xt[:, 3:6, 3 * QT : S]
            )
            nc.sync.dma_start(out=msk_sb, in_=msk[:, :, :])
            nc.scalar.dma_start(out=wo_sb, in_=wo[:, :, :])

            ones_sb = pp.tile([1, P], F16)
            nc.vector.memset(ones_sb, 1.0)

            # Dummy matmuls on the just-memset ones tile: keeps TensorE busy
            # during the initial loads so the HAM clock-gate is already warm
            # (2.4 GHz) when the first real matmul issues.
            ps_warm = psA.tile([P, QT], F32, tag="mm", name="ps_warm")
            for w in range(40):
                nc.tensor.matmul(
                    ps_warm[:, 0:P], lhsT=ones_sb, rhs=ones_sb,
                    start=True, stop=True, skip_group_check=(w > 0),
                )

            # ---- Q^T [384(3x128), S] and K^T [512(4x128), S] fp16 ----
            # (bias-add + fp16 cast on ScalarE)
            # qt chunks: 0 = h0 dims 0:128; 1 = h0 rem rows 0:64 + h1 rem
            # rows 64:128; 2 = h1 dims 0:128.
            # kt chunks: 0 = h0 main; 1 = h0 rem rows 0:64, rows 64:128 ZERO;
            # 2 = h1 main; 3 = rows 0:64 ZERO, h1 rem rows 64:128.  The zero
            # rows annihilate the other head's remainder in qt chunk 1, so
            # every score matmul contracts a full 128 rows.
            qt_sb = pp.tile([P, 3, S], F16)
            kt_sb = pp.tile([P, 4, S], F16)
            nc.vector.memset(kt_sb[64:128, 1, :], 0.0)
            nc.vector.memset(kt_sb[0:64, 3, :], 0.0)
            for s in range(NQ):
                qs = slice(s * QT, (s + 1) * QT)
                for t, (w_sb, boff) in enumerate([(wq_sb, 0), (wk_sb, 3)]):
                    for m in range(3):
                        ps = psA.tile(
                            [P, QT], F32, tag="mm", name=f"psp{t}_{m}_{s}"
                        )
                        for c in range(KC):
                            nc.tensor.matmul(
                                ps,
                                lhsT=w_sb[:, c, m * P : (m + 1) * P],
                                rhs=x_sb[:, c, qs],
                                start=(c == 0),
                                stop=(c == KC - 1),
                            )
                        if t == 0:
                            nc.scalar.activation(
                                qt_sb[:, m, qs], ps, IDENT,
                                bias=bqk_sb[:, boff + m : boff + m + 1],
                            )
                        elif m != 1:
                            nc.scalar.activation(
                                kt_sb[:, 2 * (m // 2), qs], ps, IDENT,
                                bias=bqk_sb[:, boff + m : boff + m + 1],
                            )
                        else:
                            nc.scalar.activation(
                                kt_sb[0:64, 1, qs], ps[0:64, :], IDENT,
                                bias=bqk_sb[0:64, boff + 1 : boff + 2],
                            )
                            nc.scalar.activation(
                                kt_sb[64:128, 3, qs], ps[64:128, :], IDENT,
                                bias=bqk_sb[64:128, boff + 1 : boff + 2],
                            )

            # V bias broadcast to all partitions: bvb[p, n] = bv[n]
            bvb_sb = pp.tile([P, DD], F32)
            nc.gpsimd.partition_broadcast(bvb_sb, bvr_sb)

            # ---- attention state ----
            e_store = {}   # j -> E tiles
            cx_store = {}  # j -> (cA, cB) ctx psums
            rd_store = {}  # j -> broadcast 1/denom fp16 tiles
            ct_store = {}  # j -> (ctA0, ctA1, ctM) normalized fp16 ctx

            def emit_scores_i(j, i, qs):
                """Scores + exp for key tile i of q-tile j; the causal mask
                multiply runs on Pool right after the exp."""
                ks = slice(i * P, (i + 1) * P)
                sps = [
                    psA.tile([P, QT], F32, tag="mm", name=f"sc{h}_{j}_{i}")
                    for h in range(2)
                ]
                for h in range(2):
                    nc.tensor.matmul(
                        sps[h], lhsT=kt_sb[:, 2 * h, ks], rhs=qt_sb[:, 2 * h, qs],
                        start=True, stop=False,
                    )
                    nc.tensor.matmul(
                        sps[h], lhsT=kt_sb[:, 2 * h + 1, ks], rhs=qt_sb[:, 1, qs],
                        start=False, stop=True,
                    )
                r = i - 4 * j
                pair = []
                for h in range(2):
                    e = ep.tile([P, QT], F16, tag="e", name=f"e{h}_{j}_{i}")
                    nc.scalar.activation(e, sps[h], EXP, scale=SCALE)
                    if r >= 0:
                        nc.gpsimd.tensor_mul(e, e, msk_sb[:, r, :])
                    pair.append(e)
                return pair

            # ---- V projection, seq-major, ones column at col 192 ----
            # v_sb[:, i, h, :] = [V_d0:128 | V_d128:192 | ones]; ctx piece A
            # = cols 0:128, piece B = cols 128:193 (64 V dims + denominator).
            # Query tiles are processed in DESCENDING order, so the longest
            # score phase (j=3, 16 key tiles) interleaves 1:1 with this loop.
            v_sb = pp.tile([P, NK, 2, DH + 1], F16)
            nc.gpsimd.memset(v_sb[:, :, :, DH : DH + 1], 1.0)
            ets3 = []
            qs3 = slice(3 * QT, 4 * QT)
            for i in range(NK):
                ps = psA.tile([P, QT], F32, tag="mm", name=f"psv{i}")
                for c in range(KC):
                    nc.tensor.matmul(
                        ps[:, 0:DD],
                        lhsT=x_sb[:, c, i * P : (i + 1) * P],
                        rhs=wv_sb[:, c, :],
                        start=(c == 0),
                        stop=(c == KC - 1),
                    )
                for h in range(2):
                    nc.vector.tensor_add(
                        v_sb[:, i, h, 0:DH],
                        ps[:, h * DH : (h + 1) * DH],
                        bvb_sb[:, h * DH : (h + 1) * DH],
                    )
                ets3.append(emit_scores_i(3, i, qs3))
            e_store[3] = ets3

            # ---- attention + output projection, software-pipelined ----
            def alloc_ctx(j):
                cA = [
                    psC.tile([P, QT], F32, tag="cA", name=f"cA{h}_{j}", bufs=2)
                    for h in range(2)
                ]
                cB = [
                    psC.tile([65, QT], F32, tag="cB", name=f"cB{h}_{j}", bufs=3)
                    for h in range(2)
                ]
                cx_store[j] = (cA, cB)
                return cA, cB

            def emit_recip(j):
                """Denominator row (ACT extract, fp16) -> 1/d (DVE fp16)
                -> all-partition broadcast (Pool).  Emitted right after the
                cB chains stop so the whole chain overlaps the block."""
                _, cB = cx_store[j]
                bsbs = []
                for h in range(2):
                    dn = wp.tile([1, QT], F16, tag="dn", name=f"dn{h}_{j}")
                    nc.scalar.copy(dn, cB[h][64:65, :])
                    rc = wp.tile([1, QT], F16, tag="rc", name=f"rc{h}_{j}")
                    with nc.allow_low_precision(reason="fp16 softmax denom"):
                        nc.vector.reciprocal(rc, dn)
                    bsb = wp.tile([P, QT], F16, tag="bsb", name=f"bsb{h}_{j}")
                    nc.gpsimd.partition_broadcast(bsb, rc)
                    bsbs.append(bsb)
                rd_store[j] = bsbs

            def emit_finish(j):
                """Scale ctx by the broadcast 1/denom (DVE).  The two heads'
                64-dim ctx remainders pack into one [128, QT] tile (ctM) so
                the output projection is 3 dense K=128 matmuls."""
                cA, cB = cx_store.pop(j)
                bsbs = rd_store.pop(j)
                ctA0 = cp.tile([P, QT], F16, tag="ctA0", name=f"ctA0_{j}")
                ctA1 = cp.tile([P, QT], F16, tag="ctA1", name=f"ctA1_{j}")
                ctM = cp.tile([P, QT], F16, tag="ctM", name=f"ctM_{j}")
                nc.vector.tensor_mul(ctA0, cA[0], bsbs[0])
                nc.vector.tensor_mul(ctA1, cA[1], bsbs[1])
                nc.vector.tensor_mul(ctM[0:64, :], cB[0][0:64, :], bsbs[0][0:64, :])
                nc.vector.tensor_mul(ctM[64:128, :], cB[1][0:64, :], bsbs[1][64:128, :])
                ct_store[j] = (ctA0, ctA1, ctM)

            def emit_outproj(j):
                """Output projection + fp16 store for query tile j."""
                qs = slice(j * QT, (j + 1) * QT)
                ctA0, ctA1, ctM = ct_store.pop(j)
                for m in range(KC):
                    ms = slice(m * P, (m + 1) * P)
                    po = psA.tile([P, QT], F32, tag="mm", name=f"po{m}_{j}")
                    nc.tensor.matmul(po, lhsT=wo_sb[:, 0, ms], rhs=ctA0, start=True, stop=False)
                    nc.tensor.matmul(po, lhsT=wo_sb[:, 1, ms], rhs=ctA1, start=False, stop=False)
                    nc.tensor.matmul(po, lhsT=wo_sb[:, 2, ms], rhs=ctM, start=False, stop=True)
                    osb = op_.tile([P, QT], F16, tag="osb", name=f"osb{m}_{j}")
                    nc.scalar.copy(osb, po)
                    nc.sync.dma_start(out=out_t[:, m, qs], in_=osb)

            def emit_cb_chain(jc, ec, cB):
                nk_c = 4 * jc + 4
                for i in range(nk_c):
                    for h in range(2):
                        nc.tensor.matmul(
                            cB[h],
                            lhsT=v_sb[:, i, h, P : DH + 1],
                            rhs=ec[i][h],
                            start=(i == 0),
                            stop=(i == nk_c - 1),
                        )

            def emit_ca_i(jc, ec, cA, i):
                nk_c = 4 * jc + 4
                for h in range(2):
                    nc.tensor.matmul(
                        cA[h],
                        lhsT=v_sb[:, i, h, 0:P],
                        rhs=ec[i][h],
                        start=(i == 0),
                        stop=(i == nk_c - 1),
                    )

            def emit_block(js):
                """Score phase of q-tile js with the ctx of q-tile js+1
                interleaved, plus the out-projection of q-tile js+2."""
                jc = js + 1
                nk_s = 4 * js + 4
                nk_c = 4 * jc + 4
                qs = slice(js * QT, (js + 1) * QT)
                ec = e_store.pop(jc)
                cA, cB = alloc_ctx(jc)
                emit_cb_chain(jc, ec, cB)
                emit_recip(jc)
                if jc + 1 in ct_store:
                    emit_outproj(jc + 1)
                ets = []
                for i in range(nk_s):
                    emit_ca_i(jc, ec, cA, i)
                    ets.append(emit_scores_i(js, i, qs))
                for i in range(nk_s, nk_c):
                    emit_ca_i(jc, ec, cA, i)
                emit_finish(jc)
                e_store[js] = ets

            for js in range(NQ - 2, -1, -1):  # 2, 1, 0
                emit_block(js)
            # epilogue: ctx(0) (4 key tiles) + the two pending projections.
            e0 = e_store.pop(0)
            cA, cB = alloc_ctx(0)
            emit_cb_chain(0, e0, cB)
            emit_recip(0)
            emit_outproj(1)
            for i in range(4):
                emit_ca_i(0, e0, cA, i)
            emit_finish(0)
            emit_outproj(0)

    nc.compile()
    return nc


def _get_nc():
    if "nc" not in _CACHE:
        _CACHE["nc"] = _build_nc()
    return _CACHE["nc"]


def _masks():
    kk = np.arange(P)[:, None, None]
    r = np.arange(4)[None, :, None]
    qq = np.arange(QT)[None, None, :]
    return (qq >= kk + P * r).astype(np.float16)


def host_prep(x, Wq, bq, Wk, bk, Wv, bv, Wo):
    """Build the 8 per-core input maps (core c: batch c//2, head-pair c%2)."""
    f16 = np.float16
    x = np.asarray(x, dtype=np.float32)
    Wq, Wk, Wv, Wo = (np.asarray(a, dtype=np.float32) for a in (Wq, Wk, Wv, Wo))
    bq, bk, bv = (np.asarray(a, dtype=np.float32) for a in (bq, bk, bv))
    masks = _masks()
    xt16 = {}
    for b in range(4):
        xt16[b] = np.ascontiguousarray(
            x[b].T.reshape(KC, P, S).transpose(1, 0, 2)
        ).astype(f16)
    in_maps = []
    for c in range(8):
        b, hp = divmod(c, 2)
        cs = slice(hp * DD, (hp + 1) * DD)
        wq16 = np.ascontiguousarray(
            Wq[:, cs].reshape(KC, P, DD).transpose(1, 0, 2)
        ).astype(f16)
        wk16 = np.ascontiguousarray(
            Wk[:, cs].reshape(KC, P, DD).transpose(1, 0, 2)
        ).astype(f16)
        wv16 = np.ascontiguousarray(
            Wv[:, cs].reshape(KC, P, DD).transpose(1, 0, 2)
        ).astype(f16)
        wo_s = Wo[cs, :]
        woc = np.zeros((P, 3, D), np.float32)
        woc[:, 0, :] = wo_s[0:128]      # h0 main
        woc[:, 1, :] = wo_s[192:320]    # h1 main
        woc[0:64, 2, :] = wo_s[128:192]   # h0 remainder
        woc[64:128, 2, :] = wo_s[320:384]  # h1 remainder
        bqk_c = np.concatenate(
            [bq[cs].reshape(3, P).T, bk[cs].reshape(3, P).T], axis=1
        ).astype(np.float32)
        in_maps.append(
            {
                "xt": xt16[b],
                "wq": wq16,
                "wk": wk16,
                "wv": wv16,
                "wo": woc.astype(f16),
                "bqk": np.ascontiguousarray(bqk_c),
                "bvr": np.ascontiguousarray(bv[cs].reshape(1, DD)).astype(np.float32),
                "msk": masks,
            }
        )
    return in_maps


def combine(per_core_out, bo):
    """Sum the per-batch core pairs and undo the transposed layout."""
    bo = np.asarray(bo, dtype=np.float32)
    out = np.empty((4, S, D), np.float32)
    for b in range(4):
        pt = (per_core_out[2 * b].astype(np.float32)
              + per_core_out[2 * b + 1].astype(np.float32))  # [P, KC, S]
        out[b] = pt.transpose(1, 0, 2).reshape(D, S).T + bo
    return out


def run(inp, trace=False):
    from concourse.bass_utils import run_bass_kernel_spmd

    nc = _get_nc()
    in_maps = host_prep(
        inp["inputs"], inp["Wq"], inp["bq"], inp["Wk"], inp["bk"],
        inp["Wv"], inp["bv"], inp["Wo"],
    )
    kw = {}
    if trace:
        kw = dict(trace=True, trace_cores=list(range(8)))
    res = run_bass_kernel_spmd(nc, in_maps, core_ids=list(range(8)), **kw)
    out = combine([r["out_t"] for r in res.results], inp["bo"])
    return out, res


def kernel(inputs, Wq, bq, Wk, bk, Wv, bv, Wo, bo):
    out, _ = run(
        {"inputs": inputs, "Wq": Wq, "bq": bq, "Wk": Wk, "bk": bk,
         "Wv": Wv, "bv": bv, "Wo": Wo, "bo": bo}
    )
    return out
